# revision 80
# baseline (speedup 1.0000x reference)
"""DenseCRF loss kernel for Trainium2, data-parallel over batch on 8 NeuronCores.

reference:
  seg = bilinear_resize(segmentations, 128->64)            # [N,K,64,64]
  f_i = [x_i/50, y_i/50, r_i/15, g_i/15, b_i/15]           # 5-dim bilateral feature
  W_ij = exp(-0.5*|f_i - f_j|^2)                           # [P,P], P=4096
  loss = WEIGHT * (-sum_k s_k^T W s_k) / N

Per core (1 image): W block = exp(G - q_i - q_j) with G the 5-d Gram matrix.
G is computed on the TensorEngine as a 22-row bf16 matmul where every feature is
split hi/lo into two bf16 values (bf16 products are exact in the fp32 PSUM
accumulator, so the only error is the tiny split residual). -q_i rides two bf16
aux rows; -q_j is the fp32 per-partition bias of the Exp activation. The exp'd
block (bf16) is contracted against the resized segmentation with PSUM
accumulation; a DVE multiply+reduce forms the scalar, host sums 8 cores.

Row pairing of the 22-row contraction (FA row r pairs with FB row r):
  FA: [H5 | H5 | L5 | L5 | 1 1]     H5 = [pxh pyh fh_r fh_g fh_b]
  FB: [H5 | L5 | H5 | L5 | -qh -ql] L5 = [pxl pyl fl_r fl_g fl_b]
"""

import sys

sys.path.insert(0, "/opt/trn_rl_repo")

import numpy as np
import ml_dtypes

import concourse.bass as bass
import concourse.tile as tile
from concourse import bacc, bass_isa, mybir
from concourse.bass_utils import run_bass_kernel_spmd

F32 = mybir.dt.float32
BF16 = mybir.dt.bfloat16
AF = mybir.ActivationFunctionType
ALU = mybir.AluOpType
BF = ml_dtypes.bfloat16

N, C, K = 8, 3, 21
H, W = 64, 64
P = H * W  # 4096
SIGMA_RGB = 15.0
SXY = 100.0 * 0.5  # sigma_xy * scale
WEIGHT = 1e-8
LN2 = float(np.log(2.0))
NB = 32  # 128-row chunks of P
NG = 8  # 512-col groups of P


def _resize_matrix():
    """[64,128] weights of jax.image.resize(..., method='bilinear') along one dim
    (triangle kernel, antialias=True, scale=0.5, renormalized)."""
    y = np.arange(128, dtype=np.float64)[:, None]
    sample = 2.0 * np.arange(64, dtype=np.float64)[None, :] + 0.5
    w = np.maximum(0.0, 1.0 - 0.5 * np.abs(y - sample))
    w = w / w.sum(axis=0, keepdims=True)
    return np.ascontiguousarray(w.T.astype(np.float32))  # [64,128]


def _consts():
    R = _resize_matrix()  # [64,128]
    rtf = np.ascontiguousarray(R.T)  # [128,64] f32
    rtb = rtf.astype(BF)
    idf = np.eye(128, dtype=np.float32)
    idb = idf.astype(BF)
    i = np.arange(P, dtype=np.float32)
    px = (i % 64).astype(np.float32) / np.float32(SXY)
    py = (i // 64).astype(np.float32) / np.float32(SXY)
    pos = np.stack([px, py])  # [2,P] f32
    ph2 = pos.astype(BF)
    pl2 = (pos - ph2.astype(np.float32)).astype(BF)
    pf2 = ph2.astype(np.float32) + pl2.astype(np.float32)  # exact f~ for positions
    # constant skeletons of FA/FB: position + ones rows, zeros where the
    # color / q rows get DMA'd in on-device
    fabA = np.zeros((22, P), dtype=BF)
    fabB = np.zeros((22, P), dtype=BF)
    fabA[0:2] = ph2
    fabA[5:7] = ph2
    fabA[10:12] = pl2
    fabA[15:17] = pl2
    fabA[20:22] = np.ones((2, P), dtype=BF)
    fabB[0:2] = ph2
    fabB[10:12] = ph2
    fabB[5:7] = pl2
    fabB[15:17] = pl2
    neghalf5 = np.full((5, 1), -0.5, dtype=np.float32)
    return dict(rtf=rtf, rtb=rtb, idf=idf, idb=idb, fabA=fabA, fabB=fabB,
                pf2=pf2, neghalf5=neghalf5)


def _build():
    nc = bacc.Bacc()
    images_d = nc.dram_tensor("images", [C, H, W], F32, kind="ExternalInput")
    seg_d = nc.dram_tensor("segmentations", [K, 128, 128], F32, kind="ExternalInput")
    rtf_d = nc.dram_tensor("rtf", [128, 64], F32, kind="ExternalInput")
    rtb_d = nc.dram_tensor("rtb", [128, 64], BF16, kind="ExternalInput")
    idf_d = nc.dram_tensor("idf", [128, 128], F32, kind="ExternalInput")
    idb_d = nc.dram_tensor("idb", [128, 128], BF16, kind="ExternalInput")
    fabA_d = nc.dram_tensor("fabA", [22, P], BF16, kind="ExternalInput")
    fabB_d = nc.dram_tensor("fabB", [22, P], BF16, kind="ExternalInput")
    pf2_d = nc.dram_tensor("pf2", [2, P], F32, kind="ExternalInput")
    nh5_d = nc.dram_tensor("neghalf5", [5, 1], F32, kind="ExternalInput")
    out_d = nc.dram_tensor("out", [1], F32, kind="ExternalOutput")

    with tile.TileContext(nc) as tc:
        with tc.tile_pool(name="persist", bufs=1) as pp:
            FA = pp.tile([22, P], BF16, tag="FA")
            FB = pp.tile([22, P], BF16, tag="FB")
            qcol = pp.tile([128, NB], F32, tag="qcol")
            qcol2 = pp.tile([128, NB], F32, tag="qcol2")
            Ftil = pp.tile([5, P], F32, tag="Ftil")
            Fsq = pp.tile([5, P], F32, tag="Fsq")
            q2ar = pp.tile([5, P], F32, tag="q2ar")
            fh3 = pp.tile([3, P], BF16, tag="fh3")
            fl3 = pp.tile([3, P], BF16, tag="fl3")
            qh1 = pp.tile([1, P], BF16, tag="qh1")
            ql1 = pp.tile([1, P], BF16, tag="ql1")
            nh5_s = pp.tile([5, 1], F32, tag="nh5")
            Srow = pp.tile([K, P], F32, tag="Srow")
            STt = pp.tile([128, NB * K], BF16, tag="STt")
            STt2 = pp.tile([128, NB * K], BF16, tag="STt2")
            rtf_s = pp.tile([128, 64], F32, tag="rtf")
            rtb_s = pp.tile([128, 64], BF16, tag="rtb")
            idf_s = pp.tile([128, 128], F32, tag="idf")
            idb_s = pp.tile([128, 128], BF16, tag="idb")
            img_s = pp.tile([C, P], F32, tag="img")
            seg_s = pp.tile([128, K * 128], F32, tag="seg")
            A_sb = pp.tile([64, K * 128], BF16, tag="A_sb")
            At = pp.tile([128, K * 64], BF16, tag="At")
            partials = pp.tile([K, NG], F32, tag="partials")
            pr1 = pp.tile([K, 1], F32, tag="pr1")
            tot = pp.tile([K, 1], F32, tag="tot")
            osb = pp.tile([1, 1], F32, tag="osb")

            # ---- load inputs / constants ----
            # DMA issue cost (~1.7us each) serializes per queue: spread over the
            # three DMA-capable queues. The q-chain (images -> colors -> Fsq ->
            # all-reduce -> qh/ql -> FB rows) is the critical path, so the Pool
            # queue carries only seg (the all-reduce must run early) and the
            # ACT queue runs its compute before its replica DMAs.
            dma = nc.sync.dma_start
            dmag = nc.gpsimd.dma_start
            dmaa = nc.scalar.dma_start
            inv15 = float(np.float32(1.0) / np.float32(SIGMA_RGB))
            # Queue layout: images first on the gpsimd queue (it gates the
            # q-chain), then resize inputs in usage order; FA/q-row writes ride
            # the sync queue tail; FB color replicas ride the ACT queue after
            # its compute.
            dma(img_s[:], images_d.rearrange("c h w -> c (h w)"))
            dmag(seg_s[:], seg_d.rearrange("k y x -> y k x"))
            dmag(FB[:], fabB_d[:])
            dmag(rtf_s[:], rtf_d[:])
            dmag(idb_s[:], idb_d[:])
            dmag(rtb_s[:], rtb_d[:])
            dmag(idf_s[:], idf_d[:])
            dmaa(Ftil[3:5, :], pf2_d[:])
            dma(FA[:], fabA_d[:])
            dmaa(nh5_s[:], nh5_d[:])

            # color features (hi/lo split of img/15) at partition 0; engines
            # cannot address partition offsets that aren't multiples of 32, so
            # rows are staged and DMA'd into the FA/FB row slots.
            inv15 = float(np.float32(1.0) / np.float32(SIGMA_RGB))
            # q-chain steps split into ACT/DVE halves so the two engines run
            # the serial prep in parallel (the chain gates the main loop)
            Hm = P // 2
            nc.scalar.activation(fh3[:, 0:Hm], img_s[:, 0:Hm], AF.Copy,
                                 scale=inv15)
            nc.vector.tensor_scalar_mul(fh3[:, Hm:], img_s[:, Hm:], inv15)
            nc.scalar.activation(Ftil[0:3, 0:Hm], img_s[:, 0:Hm], AF.Copy,
                                 scale=inv15)
            nc.vector.tensor_scalar_mul(Ftil[0:3, Hm:], img_s[:, Hm:], inv15)
            # q = 0.5|f~|^2  (Ftil rows: [colors | positions]; sum is order-free)
            nc.vector.scalar_tensor_tensor(
                fl3[:], img_s[:], inv15, fh3[:], ALU.mult, ALU.subtract
            )  # fl = img/15 - fh
            nc.scalar.activation(Fsq[:, 0:Hm], Ftil[:, 0:Hm], AF.Square)
            nc.vector.tensor_mul(Fsq[:, Hm:], Ftil[:, Hm:], Ftil[:, Hm:])
            nc.gpsimd.partition_all_reduce(q2ar[:], Fsq[:], 5, bass_isa.ReduceOp.add)
            q2row = q2ar[0:1, :]
            nc.scalar.activation(qh1[:, 0:Hm], q2row[:, 0:Hm], AF.Copy,
                                 scale=-0.5)
            nc.vector.tensor_scalar_mul(qh1[:, Hm:], q2row[:, Hm:], -0.5)
            nc.vector.scalar_tensor_tensor(
                ql1[:], q2row, -0.5, qh1[:], ALU.mult, ALU.subtract
            )  # -ql = -q - (-qh)
            dma(FB[20:21, :], qh1[:])
            dma(FB[21:22, :], ql1[:])
            dmaa(FB[2:5, :], fh3[:])
            dmaa(FB[12:15, :], fh3[:])
            dmaa(FB[7:10, :], fl3[:])
            dmaa(FB[17:20, :], fl3[:])
            dma(FA[2:5, :], fh3[:])
            dma(FA[7:10, :], fh3[:])
            dma(FA[12:15, :], fl3[:])
            dma(FA[17:20, :], fl3[:])

            with tc.tile_pool(name="prep_ps", bufs=8, space="PSUM") as pps:
                # qcol[:, b] = -q for chunk b (fp32, used as Exp bias)
                qps = pps.tile([128, NB], F32, tag="ps", name="qps")
                for b in range(NB):
                    nc.tensor.matmul(
                        qps[:, b : b + 1],
                        Fsq[:, b * 128 : (b + 1) * 128],
                        nh5_s[:],
                        start=True, stop=True,
                    )
                nc.vector.tensor_copy(qcol[:], qps[:])
                nc.vector.tensor_scalar_add(qcol2[:], qcol[:], LN2)
                # ---- resize: rows (contract Y) ----
                # (emitted before the q-dependent matmuls: PE executes in order,
                # and resize inputs arrive long before Fsq is ready)
                for c0 in range(0, K * 128, 512):
                    c1 = min(c0 + 512, K * 128)
                    aps = pps.tile([64, 512], F32, tag="ps", name=f"aps{c0}")
                    nc.tensor.matmul(
                        aps[:, : c1 - c0], rtf_s[:], seg_s[:, c0:c1],
                        start=True, stop=True,
                    )
                    nc.vector.tensor_copy(A_sb[:, c0:c1], aps[:, : c1 - c0])
                # transpose per class: At[X, (k,y')]
                for k0 in range(0, K, 8):
                    k1 = min(k0 + 8, K)
                    tps = pps.tile([128, 64 * 8], BF16, tag="ps", name=f"tps{k0}")
                    for k in range(k0, k1):
                        nc.tensor.transpose(
                            tps[:, (k - k0) * 64 : (k - k0 + 1) * 64],
                            A_sb[0:64, k * 128 : (k + 1) * 128], idb_s[0:64, 0:64]
                        )
                    nc.vector.tensor_copy(
                        At[:, k0 * 64 : k1 * 64], tps[:, : (k1 - k0) * 64]
                    )
                # cols (contract X): Srow[k, y'*64+x']
                at3 = At[:, :].rearrange("x (k y) -> x k y", k=K, y=64)
                for yb in range(8):
                    sps = pps.tile([K, 512], F32, tag="ps", name=f"sps{yb}")
                    for yl in range(8):
                        yp = yb * 8 + yl
                        nc.tensor.matmul(
                            sps[:, yl * 64 : (yl + 1) * 64],
                            at3[:, :, yp], rtb_s[:],
                            start=True, stop=True,
                        )
                    nc.vector.tensor_copy(Srow[:, yb * 512 : (yb + 1) * 512], sps[:])
                # STt chunks: [128, 21] per b (bf16, acc-matmul weights)
                for b0 in range(0, NB, 8):
                    t2 = pps.tile([128, K * 8], F32, tag="ps", name=f"t2_{b0}")
                    for b in range(b0, b0 + 8):
                        nc.tensor.transpose(
                            t2[:, (b - b0) * K : (b - b0 + 1) * K],
                            Srow[:, b * 128 : (b + 1) * 128], idf_s[0:K, 0:K]
                        )
                    nc.vector.tensor_copy(STt[:, b0 * K : (b0 + 8) * K], t2[:])

                nc.scalar.activation(STt2[:], STt[:], AF.Copy, scale=2.0)


            # ---- main loop: 4 passes x 32 chunks x one 1024-wide exp unit ----
            with (
                tc.tile_pool(name="gps", bufs=3, space="PSUM") as gps,
                tc.tile_pool(name="accps", bufs=2, space="PSUM") as accps,
                tc.tile_pool(name="ep", bufs=8) as ep,
                tc.tile_pool(name="finp", bufs=2) as finp,
            ):
                for p in range(4):
                    accs = [
                        accps.tile([K, 512], F32, tag="acc", name=f"acc{p}_{gi}")
                        for gi in range(2)
                    ]
                    last = (8 * p + 3, 8 * p + 7)
                    pend = []  # software pipeline: acc-matmuls lag one chunk

                    def flush_one(accs=accs, last=last, pend=pend):
                        pb, pw, ptg, pet = pend.pop(0)
                        for gl, stt in ptg:
                            sl = pet[:, gl * 512:(gl + 1) * 512] if pw == 2 \
                                else pet[:, 0:512]
                            nc.tensor.matmul(
                                accs[gl][:], stt[:, pb * K:(pb + 1) * K], sl,
                                start=(pb == 0), stop=(pb == last[gl]),
                            )

                    # W is symmetric: only chunks touching the diagonal or the
                    # upper triangle of this pass's column pair are computed.
                    # Strictly-upper blocks carry factor 2 via qcol2 (= qcol +
                    # ln2) or the doubled seg-transpose STt2.
                    for b in range(8 * p + 8):
                        if b < 8 * p:
                            w, bias_t, tg = 2, qcol2, [(0, STt), (1, STt)]
                        elif b < 8 * p + 4:
                            w, bias_t, tg = 2, qcol, [(0, STt), (1, STt2)]
                        else:
                            w, bias_t, tg = 1, qcol, [(1, STt)]
                        if len(pend) > 1:
                            flush_one()
                        fa_b = FA[:, b * 128 : (b + 1) * 128]
                        g0 = p * 2
                        gt = gps.tile([128, 1024], F32, tag="g", name=f"g{p}_{b}")
                        if w == 2:
                            nc.tensor.matmul(
                                gt[:, 0:512], fa_b,
                                FB[:, g0 * 512 : (g0 + 1) * 512],
                                start=True, stop=True,
                            )
                            nc.tensor.matmul(
                                gt[:, 512:1024], fa_b,
                                FB[:, (g0 + 1) * 512 : (g0 + 2) * 512],
                                start=True, stop=True,
                            )
                        else:
                            nc.tensor.matmul(
                                gt[:, 0:512], fa_b,
                                FB[:, (g0 + 1) * 512 : (g0 + 2) * 512],
                                start=True, stop=True,
                            )
                        et = ep.tile([128, 1024], BF16, tag="e", name=f"e{p}_{b}")
                        nc.scalar.activation(
                            et[:, 0:w * 512], gt[:, 0:w * 512], AF.Exp,
                            bias=bias_t[:, b : b + 1]
                        )
                        pend.append((b, w, tg, et))
                    while pend:
                        flush_one()
                    # loss partials: sum_k,i acc[k,i] * Srow[k,i]
                    for gi in range(2):
                        g = p * 2 + gi
                        sc = finp.tile([K, 512], F32, tag="sc", name=f"sc{p}_{gi}")
                        nc.vector.tensor_mul(
                            sc[:], accs[gi][:], Srow[:, g * 512 : (g + 1) * 512]
                        )
                        nc.vector.tensor_reduce(
                            partials[:, g : g + 1], sc[:], mybir.AxisListType.X, ALU.add
                        )

                nc.vector.tensor_reduce(pr1[:], partials[:], mybir.AxisListType.X, ALU.add)
                nc.gpsimd.partition_all_reduce(tot[:], pr1[:], K, bass_isa.ReduceOp.add)
                nc.scalar.activation(osb[:], tot[0:1, :], AF.Copy, scale=float(-WEIGHT / N))
                nc.sync.dma_start(out_d[:], osb[:])

    nc.finalize()
    return nc


_CACHE = {}


def _get_nc():
    if "nc" not in _CACHE:
        _CACHE["nc"] = _build()
    return _CACHE["nc"]


def kernel(images: np.ndarray, segmentations: np.ndarray) -> np.ndarray:
    images = np.ascontiguousarray(np.asarray(images, dtype=np.float32))
    segmentations = np.ascontiguousarray(np.asarray(segmentations, dtype=np.float32))
    assert images.shape == (N, C, H, W) and segmentations.shape == (N, K, 128, 128)
    nc = _get_nc()
    consts = _consts()
    in_maps = [
        {"images": images[n], "segmentations": segmentations[n], **consts}
        for n in range(N)
    ]
    res = run_bass_kernel_spmd(nc, in_maps, list(range(N)))
    total = sum(float(res.results[n]["out"][0]) for n in range(N))
    return np.array([total], dtype=np.float32)


if __name__ == "__main__":
    rng = np.random.RandomState(0)
    img = rng.rand(N, C, H, W).astype(np.float32) * 255.0
    seg = rng.rand(N, K, 128, 128).astype(np.float32)
    print(kernel(img, seg))



# revision 81
# speedup vs baseline: 1.0129x; 1.0129x over previous
"""DenseCRF loss kernel for Trainium2, data-parallel over batch on 8 NeuronCores.

reference:
  seg = bilinear_resize(segmentations, 128->64)            # [N,K,64,64]
  f_i = [x_i/50, y_i/50, r_i/15, g_i/15, b_i/15]           # 5-dim bilateral feature
  W_ij = exp(-0.5*|f_i - f_j|^2)                           # [P,P], P=4096
  loss = WEIGHT * (-sum_k s_k^T W s_k) / N

Per core (1 image): W block = exp(G - q_i - q_j) with G the 5-d Gram matrix.
G is computed on the TensorEngine as a 22-row bf16 matmul where every feature is
split hi/lo into two bf16 values (bf16 products are exact in the fp32 PSUM
accumulator, so the only error is the tiny split residual). -q_i rides two bf16
aux rows; -q_j is the fp32 per-partition bias of the Exp activation. The exp'd
block (bf16) is contracted against the resized segmentation with PSUM
accumulation; a DVE multiply+reduce forms the scalar, host sums 8 cores.

Row pairing of the 22-row contraction (FA row r pairs with FB row r):
  FA: [H5 | H5 | L5 | L5 | 1 1]     H5 = [pxh pyh fh_r fh_g fh_b]
  FB: [H5 | L5 | H5 | L5 | -qh -ql] L5 = [pxl pyl fl_r fl_g fl_b]
"""

import sys

sys.path.insert(0, "/opt/trn_rl_repo")

import numpy as np
import ml_dtypes

import concourse.bass as bass
import concourse.tile as tile
from concourse import bacc, bass_isa, mybir
from concourse.bass_utils import run_bass_kernel_spmd

F32 = mybir.dt.float32
BF16 = mybir.dt.bfloat16
AF = mybir.ActivationFunctionType
ALU = mybir.AluOpType
BF = ml_dtypes.bfloat16

N, C, K = 8, 3, 21
H, W = 64, 64
P = H * W  # 4096
SIGMA_RGB = 15.0
SXY = 100.0 * 0.5  # sigma_xy * scale
WEIGHT = 1e-8
LN2 = float(np.log(2.0))
NB = 32  # 128-row chunks of P
NG = 8  # 512-col groups of P


def _resize_matrix():
    """[64,128] weights of jax.image.resize(..., method='bilinear') along one dim
    (triangle kernel, antialias=True, scale=0.5, renormalized)."""
    y = np.arange(128, dtype=np.float64)[:, None]
    sample = 2.0 * np.arange(64, dtype=np.float64)[None, :] + 0.5
    w = np.maximum(0.0, 1.0 - 0.5 * np.abs(y - sample))
    w = w / w.sum(axis=0, keepdims=True)
    return np.ascontiguousarray(w.T.astype(np.float32))  # [64,128]


def _consts():
    R = _resize_matrix()  # [64,128]
    rtf = np.ascontiguousarray(R.T)  # [128,64] f32
    rtb = rtf.astype(BF)
    idf = np.eye(128, dtype=np.float32)
    idb = idf.astype(BF)
    i = np.arange(P, dtype=np.float32)
    px = (i % 64).astype(np.float32) / np.float32(SXY)
    py = (i // 64).astype(np.float32) / np.float32(SXY)
    pos = np.stack([px, py])  # [2,P] f32
    ph2 = pos.astype(BF)
    pl2 = (pos - ph2.astype(np.float32)).astype(BF)
    pf2 = ph2.astype(np.float32) + pl2.astype(np.float32)  # exact f~ for positions
    # constant skeletons of FA/FB: position + ones rows, zeros where the
    # color / q rows get DMA'd in on-device
    fabA = np.zeros((22, P), dtype=BF)
    fabB = np.zeros((22, P), dtype=BF)
    fabA[0:2] = ph2
    fabA[5:7] = ph2
    fabA[10:12] = pl2
    fabA[15:17] = pl2
    fabA[20:22] = np.ones((2, P), dtype=BF)
    fabB[0:2] = ph2
    fabB[10:12] = ph2
    fabB[5:7] = pl2
    fabB[15:17] = pl2
    neghalf5 = np.full((5, 1), -0.5, dtype=np.float32)
    return dict(rtf=rtf, rtb=rtb, idf=idf, idb=idb, fabA=fabA, fabB=fabB,
                pf2=pf2, neghalf5=neghalf5)


def _build():
    nc = bacc.Bacc()
    images_d = nc.dram_tensor("images", [C, H, W], F32, kind="ExternalInput")
    seg_d = nc.dram_tensor("segmentations", [K, 128, 128], F32, kind="ExternalInput")
    rtf_d = nc.dram_tensor("rtf", [128, 64], F32, kind="ExternalInput")
    rtb_d = nc.dram_tensor("rtb", [128, 64], BF16, kind="ExternalInput")
    idf_d = nc.dram_tensor("idf", [128, 128], F32, kind="ExternalInput")
    idb_d = nc.dram_tensor("idb", [128, 128], BF16, kind="ExternalInput")
    fabA_d = nc.dram_tensor("fabA", [22, P], BF16, kind="ExternalInput")
    fabB_d = nc.dram_tensor("fabB", [22, P], BF16, kind="ExternalInput")
    pf2_d = nc.dram_tensor("pf2", [2, P], F32, kind="ExternalInput")
    nh5_d = nc.dram_tensor("neghalf5", [5, 1], F32, kind="ExternalInput")
    out_d = nc.dram_tensor("out", [1], F32, kind="ExternalOutput")

    with tile.TileContext(nc) as tc:
        with tc.tile_pool(name="persist", bufs=1) as pp:
            FA = pp.tile([22, P], BF16, tag="FA")
            FB = pp.tile([22, P], BF16, tag="FB")
            qcol = pp.tile([128, NB], F32, tag="qcol")
            qcol2 = pp.tile([128, NB], F32, tag="qcol2")
            Ftil = pp.tile([5, P], F32, tag="Ftil")
            Fsq = pp.tile([5, P], F32, tag="Fsq")
            q2ar = pp.tile([5, P], F32, tag="q2ar")
            fh3 = pp.tile([3, P], BF16, tag="fh3")
            fl3 = pp.tile([3, P], BF16, tag="fl3")
            qh1 = pp.tile([1, P], BF16, tag="qh1")
            ql1 = pp.tile([1, P], BF16, tag="ql1")
            nh5_s = pp.tile([5, 1], F32, tag="nh5")
            Srow = pp.tile([K, P], F32, tag="Srow")
            STt = pp.tile([128, NB * K], BF16, tag="STt")
            STt2 = pp.tile([128, NB * K], BF16, tag="STt2")
            rtf_s = pp.tile([128, 64], F32, tag="rtf")
            rtb_s = pp.tile([128, 64], BF16, tag="rtb")
            idf_s = pp.tile([128, 128], F32, tag="idf")
            idb_s = pp.tile([128, 128], BF16, tag="idb")
            img_s = pp.tile([C, P], F32, tag="img")
            seg_s = pp.tile([128, K * 128], F32, tag="seg")
            A_sb = pp.tile([64, K * 128], BF16, tag="A_sb")
            At = pp.tile([128, K * 64], BF16, tag="At")
            partials = pp.tile([K, NG], F32, tag="partials")
            pr1 = pp.tile([K, 1], F32, tag="pr1")
            tot = pp.tile([K, 1], F32, tag="tot")
            osb = pp.tile([1, 1], F32, tag="osb")

            # ---- load inputs / constants ----
            # DMA issue cost (~1.7us each) serializes per queue: spread over the
            # three DMA-capable queues. The q-chain (images -> colors -> Fsq ->
            # all-reduce -> qh/ql -> FB rows) is the critical path, so the Pool
            # queue carries only seg (the all-reduce must run early) and the
            # ACT queue runs its compute before its replica DMAs.
            dma = nc.sync.dma_start
            dmag = nc.gpsimd.dma_start
            dmaa = nc.scalar.dma_start
            inv15 = float(np.float32(1.0) / np.float32(SIGMA_RGB))
            # Queue layout: images first on the gpsimd queue (it gates the
            # q-chain), then resize inputs in usage order; FA/q-row writes ride
            # the sync queue tail; FB color replicas ride the ACT queue after
            # its compute.
            dmag(img_s[:], images_d.rearrange("c h w -> c (h w)"))
            dmag(seg_s[:], seg_d.rearrange("k y x -> y k x"))
            dmag(FB[:], fabB_d[:])
            dmag(rtf_s[:], rtf_d[:])
            dmag(idb_s[:], idb_d[:])
            dmag(rtb_s[:], rtb_d[:])
            dmag(idf_s[:], idf_d[:])
            dma(Ftil[3:5, :], pf2_d[:])
            dma(FA[:], fabA_d[:])
            dma(nh5_s[:], nh5_d[:])

            # color features (hi/lo split of img/15) at partition 0; engines
            # cannot address partition offsets that aren't multiples of 32, so
            # rows are staged and DMA'd into the FA/FB row slots.
            inv15 = float(np.float32(1.0) / np.float32(SIGMA_RGB))
            # q-chain steps split into ACT/DVE halves so the two engines run
            # the serial prep in parallel (the chain gates the main loop)
            Hm = P // 2
            nc.scalar.activation(fh3[:, 0:Hm], img_s[:, 0:Hm], AF.Copy,
                                 scale=inv15)
            nc.vector.tensor_scalar_mul(fh3[:, Hm:], img_s[:, Hm:], inv15)
            nc.scalar.activation(Ftil[0:3, 0:Hm], img_s[:, 0:Hm], AF.Copy,
                                 scale=inv15)
            nc.vector.tensor_scalar_mul(Ftil[0:3, Hm:], img_s[:, Hm:], inv15)
            # q = 0.5|f~|^2  (Ftil rows: [colors | positions]; sum is order-free)
            nc.vector.scalar_tensor_tensor(
                fl3[:], img_s[:], inv15, fh3[:], ALU.mult, ALU.subtract
            )  # fl = img/15 - fh
            nc.scalar.activation(Fsq[:, 0:Hm], Ftil[:, 0:Hm], AF.Square)
            nc.vector.tensor_mul(Fsq[:, Hm:], Ftil[:, Hm:], Ftil[:, Hm:])
            nc.gpsimd.partition_all_reduce(q2ar[:], Fsq[:], 5, bass_isa.ReduceOp.add)
            q2row = q2ar[0:1, :]
            nc.scalar.activation(qh1[:, 0:Hm], q2row[:, 0:Hm], AF.Copy,
                                 scale=-0.5)
            nc.vector.tensor_scalar_mul(qh1[:, Hm:], q2row[:, Hm:], -0.5)
            nc.vector.scalar_tensor_tensor(
                ql1[:], q2row, -0.5, qh1[:], ALU.mult, ALU.subtract
            )  # -ql = -q - (-qh)
            dma(FB[20:21, :], qh1[:])
            dma(FB[21:22, :], ql1[:])
            dmaa(FB[2:5, :], fh3[:])
            dmaa(FB[12:15, :], fh3[:])
            dmaa(FB[7:10, :], fl3[:])
            dmaa(FB[17:20, :], fl3[:])
            dma(FA[2:5, :], fh3[:])
            dma(FA[7:10, :], fh3[:])
            dma(FA[12:15, :], fl3[:])
            dma(FA[17:20, :], fl3[:])

            with tc.tile_pool(name="prep_ps", bufs=8, space="PSUM") as pps:
                # qcol[:, b] = -q for chunk b (fp32, used as Exp bias)
                qps = pps.tile([128, NB], F32, tag="ps", name="qps")
                for b in range(NB):
                    nc.tensor.matmul(
                        qps[:, b : b + 1],
                        Fsq[:, b * 128 : (b + 1) * 128],
                        nh5_s[:],
                        start=True, stop=True,
                    )
                nc.vector.tensor_copy(qcol[:], qps[:])
                nc.vector.tensor_scalar_add(qcol2[:], qcol[:], LN2)
                # ---- resize: rows (contract Y) ----
                # (emitted before the q-dependent matmuls: PE executes in order,
                # and resize inputs arrive long before Fsq is ready)
                for c0 in range(0, K * 128, 512):
                    c1 = min(c0 + 512, K * 128)
                    aps = pps.tile([64, 512], F32, tag="ps", name=f"aps{c0}")
                    nc.tensor.matmul(
                        aps[:, : c1 - c0], rtf_s[:], seg_s[:, c0:c1],
                        start=True, stop=True,
                    )
                    nc.vector.tensor_copy(A_sb[:, c0:c1], aps[:, : c1 - c0])
                # transpose per class: At[X, (k,y')]
                for k0 in range(0, K, 8):
                    k1 = min(k0 + 8, K)
                    tps = pps.tile([128, 64 * 8], BF16, tag="ps", name=f"tps{k0}")
                    for k in range(k0, k1):
                        nc.tensor.transpose(
                            tps[:, (k - k0) * 64 : (k - k0 + 1) * 64],
                            A_sb[0:64, k * 128 : (k + 1) * 128], idb_s[0:64, 0:64]
                        )
                    nc.vector.tensor_copy(
                        At[:, k0 * 64 : k1 * 64], tps[:, : (k1 - k0) * 64]
                    )
                # cols (contract X): Srow[k, y'*64+x']
                at3 = At[:, :].rearrange("x (k y) -> x k y", k=K, y=64)
                for yb in range(8):
                    sps = pps.tile([K, 512], F32, tag="ps", name=f"sps{yb}")
                    for yl in range(8):
                        yp = yb * 8 + yl
                        nc.tensor.matmul(
                            sps[:, yl * 64 : (yl + 1) * 64],
                            at3[:, :, yp], rtb_s[:],
                            start=True, stop=True,
                        )
                    nc.vector.tensor_copy(Srow[:, yb * 512 : (yb + 1) * 512], sps[:])
                # STt chunks: [128, 21] per b (bf16, acc-matmul weights)
                for b0 in range(0, NB, 8):
                    t2 = pps.tile([128, K * 8], F32, tag="ps", name=f"t2_{b0}")
                    for b in range(b0, b0 + 8):
                        nc.tensor.transpose(
                            t2[:, (b - b0) * K : (b - b0 + 1) * K],
                            Srow[:, b * 128 : (b + 1) * 128], idf_s[0:K, 0:K]
                        )
                    nc.vector.tensor_copy(STt[:, b0 * K : (b0 + 8) * K], t2[:])

                nc.scalar.activation(STt2[:], STt[:], AF.Copy, scale=2.0)


            # ---- main loop: 4 passes x 32 chunks x one 1024-wide exp unit ----
            with (
                tc.tile_pool(name="gps", bufs=3, space="PSUM") as gps,
                tc.tile_pool(name="accps", bufs=2, space="PSUM") as accps,
                tc.tile_pool(name="ep", bufs=8) as ep,
                tc.tile_pool(name="finp", bufs=2) as finp,
            ):
                for p in range(4):
                    accs = [
                        accps.tile([K, 512], F32, tag="acc", name=f"acc{p}_{gi}")
                        for gi in range(2)
                    ]
                    last = (8 * p + 3, 8 * p + 7)
                    pend = []  # software pipeline: acc-matmuls lag one chunk

                    def flush_one(accs=accs, last=last, pend=pend):
                        pb, pw, ptg, pet = pend.pop(0)
                        for gl, stt in ptg:
                            sl = pet[:, gl * 512:(gl + 1) * 512] if pw == 2 \
                                else pet[:, 0:512]
                            nc.tensor.matmul(
                                accs[gl][:], stt[:, pb * K:(pb + 1) * K], sl,
                                start=(pb == 0), stop=(pb == last[gl]),
                            )

                    # W is symmetric: only chunks touching the diagonal or the
                    # upper triangle of this pass's column pair are computed.
                    # Strictly-upper blocks carry factor 2 via qcol2 (= qcol +
                    # ln2) or the doubled seg-transpose STt2.
                    for b in range(8 * p + 8):
                        if b < 8 * p:
                            w, bias_t, tg = 2, qcol2, [(0, STt), (1, STt)]
                        elif b < 8 * p + 4:
                            w, bias_t, tg = 2, qcol, [(0, STt), (1, STt2)]
                        else:
                            w, bias_t, tg = 1, qcol, [(1, STt)]
                        if len(pend) > 1:
                            flush_one()
                        fa_b = FA[:, b * 128 : (b + 1) * 128]
                        g0 = p * 2
                        gt = gps.tile([128, 1024], F32, tag="g", name=f"g{p}_{b}")
                        if w == 2:
                            nc.tensor.matmul(
                                gt[:, 0:512], fa_b,
                                FB[:, g0 * 512 : (g0 + 1) * 512],
                                start=True, stop=True,
                            )
                            nc.tensor.matmul(
                                gt[:, 512:1024], fa_b,
                                FB[:, (g0 + 1) * 512 : (g0 + 2) * 512],
                                start=True, stop=True,
                            )
                        else:
                            nc.tensor.matmul(
                                gt[:, 0:512], fa_b,
                                FB[:, (g0 + 1) * 512 : (g0 + 2) * 512],
                                start=True, stop=True,
                            )
                        et = ep.tile([128, 1024], BF16, tag="e", name=f"e{p}_{b}")
                        nc.scalar.activation(
                            et[:, 0:w * 512], gt[:, 0:w * 512], AF.Exp,
                            bias=bias_t[:, b : b + 1]
                        )
                        pend.append((b, w, tg, et))
                    while pend:
                        flush_one()
                    # loss partials: sum_k,i acc[k,i] * Srow[k,i]
                    for gi in range(2):
                        g = p * 2 + gi
                        sc = finp.tile([K, 512], F32, tag="sc", name=f"sc{p}_{gi}")
                        nc.vector.tensor_mul(
                            sc[:], accs[gi][:], Srow[:, g * 512 : (g + 1) * 512]
                        )
                        nc.vector.tensor_reduce(
                            partials[:, g : g + 1], sc[:], mybir.AxisListType.X, ALU.add
                        )

                nc.vector.tensor_reduce(pr1[:], partials[:], mybir.AxisListType.X, ALU.add)
                nc.gpsimd.partition_all_reduce(tot[:], pr1[:], K, bass_isa.ReduceOp.add)
                nc.scalar.activation(osb[:], tot[0:1, :], AF.Copy, scale=float(-WEIGHT / N))
                nc.sync.dma_start(out_d[:], osb[:])

    nc.finalize()
    return nc


_CACHE = {}


def _get_nc():
    if "nc" not in _CACHE:
        _CACHE["nc"] = _build()
    return _CACHE["nc"]


def kernel(images: np.ndarray, segmentations: np.ndarray) -> np.ndarray:
    images = np.ascontiguousarray(np.asarray(images, dtype=np.float32))
    segmentations = np.ascontiguousarray(np.asarray(segmentations, dtype=np.float32))
    assert images.shape == (N, C, H, W) and segmentations.shape == (N, K, 128, 128)
    nc = _get_nc()
    consts = _consts()
    in_maps = [
        {"images": images[n], "segmentations": segmentations[n], **consts}
        for n in range(N)
    ]
    res = run_bass_kernel_spmd(nc, in_maps, list(range(N)))
    total = sum(float(res.results[n]["out"][0]) for n in range(N))
    return np.array([total], dtype=np.float32)


if __name__ == "__main__":
    rng = np.random.RandomState(0)
    img = rng.rand(N, C, H, W).astype(np.float32) * 255.0
    seg = rng.rand(N, K, 128, 128).astype(np.float32)
    print(kernel(img, seg))

